# revision 1
# baseline (speedup 1.0000x reference)
"""VQ codebook lookup kernel for Trainium2 (8 NeuronCores, data-parallel).

out[b] = emb[argmin_k ||x[b] - emb[k]||^2]

Per core (8192 rows of x):
  score[b,k] = 2*x.e_k - |e_k|^2  (argmax == argmin of distance)
  computed on the PE in three fp16 hi/lo passes (x = xh+xl, 2e^T = eh+el,
  s = xh.eh + xh.el + xl.eh) which preserves fp32-grade precision, with
  -|e_k|^2 folded in as a K=3 bias matmul (ones3^T @ [q1;q2;q3]).
  argmax via DVE max/max_index on PSUM, then indirect-DMA gather of emb rows.
"""
import os
import sys

import numpy as np

for _p in ("/opt/trn_rl_repo", "/root/.axon_site/_ro/trn_rl_repo"):
    if os.path.isdir(_p) and _p not in sys.path:
        sys.path.append(_p)

import concourse.bass as bass
import concourse.tile as tile
from concourse import bacc, mybir
from concourse.bass_utils import run_bass_kernel_spmd

N_CORES = 8
B, D, K = 65536, 256, 1024
BC = B // N_CORES            # rows per core
TILE_B = 128
N_TILES = BC // TILE_B

f32 = mybir.dt.float32
f16 = mybir.dt.float16
u32 = mybir.dt.uint32

_nc_cache = {}


def _build():
    if "nc" in _nc_cache:
        return _nc_cache["nc"]
    nc = bacc.Bacc()

    xh0 = nc.declare_dram_parameter("xh0", [128, BC], f16, isOutput=False)
    xh1 = nc.declare_dram_parameter("xh1", [128, BC], f16, isOutput=False)
    xl0 = nc.declare_dram_parameter("xl0", [128, BC], f16, isOutput=False)
    xl1 = nc.declare_dram_parameter("xl1", [128, BC], f16, isOutput=False)
    eh0 = nc.declare_dram_parameter("eh0", [128, K], f16, isOutput=False)
    eh1 = nc.declare_dram_parameter("eh1", [128, K], f16, isOutput=False)
    el0 = nc.declare_dram_parameter("el0", [128, K], f16, isOutput=False)
    el1 = nc.declare_dram_parameter("el1", [128, K], f16, isOutput=False)
    biasq = nc.declare_dram_parameter("biasq", [3, K], f16, isOutput=False)
    ones3 = nc.declare_dram_parameter("ones3", [3, 128], f16, isOutput=False)
    emb = nc.declare_dram_parameter("emb", [K, D], f32, isOutput=False)
    out = nc.declare_dram_parameter("out", [BC, D], f32, isOutput=True)

    with tile.TileContext(nc) as tc:
        with tc.tile_pool(name="res", bufs=1) as res, \
             tc.tile_pool(name="wrk", bufs=4) as wrk, \
             tc.tile_pool(name="ps", bufs=3, space="PSUM") as ps:
            txh0 = res.tile([128, BC], f16, tag="xh0")
            txh1 = res.tile([128, BC], f16, tag="xh1")
            txl0 = res.tile([128, BC], f16, tag="xl0")
            txl1 = res.tile([128, BC], f16, tag="xl1")
            teh0 = res.tile([128, K], f16, tag="eh0")
            teh1 = res.tile([128, K], f16, tag="eh1")
            tel0 = res.tile([128, K], f16, tag="el0")
            tel1 = res.tile([128, K], f16, tag="el1")
            tbias = res.tile([3, K], f16, tag="biasq")
            tones = res.tile([3, 128], f16, tag="ones3")
            for dst, src in [(txh0, xh0), (txh1, xh1), (txl0, xl0), (txl1, xl1),
                             (teh0, eh0), (teh1, eh1), (tel0, el0), (tel1, el1),
                             (tbias, biasq), (tones, ones3)]:
                nc.sync.dma_start(dst[:], src[:])

            for i in range(N_TILES):
                s = slice(i * TILE_B, (i + 1) * TILE_B)
                psc = ps.tile([128, K], f32, tag="scores")
                for h in range(2):
                    ks = slice(h * 512, (h + 1) * 512)
                    po = psc[:, ks]
                    nc.tensor.matmul(po, lhsT=tones[:], rhs=tbias[:, ks],
                                     start=True, stop=False)
                    nc.tensor.matmul(po, lhsT=txh0[:, s], rhs=teh0[:, ks],
                                     start=False, stop=False)
                    nc.tensor.matmul(po, lhsT=txh1[:, s], rhs=teh1[:, ks],
                                     start=False, stop=False)
                    nc.tensor.matmul(po, lhsT=txh0[:, s], rhs=tel0[:, ks],
                                     start=False, stop=False)
                    nc.tensor.matmul(po, lhsT=txh1[:, s], rhs=tel1[:, ks],
                                     start=False, stop=False)
                    nc.tensor.matmul(po, lhsT=txl0[:, s], rhs=teh0[:, ks],
                                     start=False, stop=False)
                    nc.tensor.matmul(po, lhsT=txl1[:, s], rhs=teh1[:, ks],
                                     start=False, stop=True)

                tmax = wrk.tile([128, 8], f32, tag="maxv")
                tidx = wrk.tile([128, 8], u32, tag="idx")
                nc.vector.max(out=tmax[:], in_=psc[:])
                nc.vector.max_index(out=tidx[:], in_max=tmax[:], in_values=psc[:])

                tg = wrk.tile([128, D], f32, tag="gat")
                nc.gpsimd.indirect_dma_start(
                    out=tg[:],
                    out_offset=None,
                    in_=emb[:],
                    in_offset=bass.IndirectOffsetOnAxis(ap=tidx[:, 0:1], axis=0),
                )
                nc.sync.dma_start(out[s, :], tg[:])

    nc.compile()
    _nc_cache["nc"] = nc
    return nc


def _prepare_inputs(x, emb):
    x = np.ascontiguousarray(np.asarray(x, dtype=np.float32))
    emb = np.ascontiguousarray(np.asarray(emb, dtype=np.float32))

    e2 = np.ascontiguousarray(2.0 * emb.T)              # [D, K] f32, exact
    eh = e2.astype(np.float16)
    el = (e2 - eh.astype(np.float32)).astype(np.float16)

    esq = (emb.astype(np.float64) ** 2).sum(axis=1)
    q = (-esq).astype(np.float32)
    q1 = q.astype(np.float16)
    r = q - q1.astype(np.float32)
    q2 = r.astype(np.float16)
    q3 = (r - q2.astype(np.float32)).astype(np.float16)
    biasq = np.ascontiguousarray(np.stack([q1, q2, q3]))  # [3, K] f16

    xh = x.astype(np.float16)
    xl = (x - xh.astype(np.float32)).astype(np.float16)
    xhT = np.ascontiguousarray(xh.T)                    # [D, B] f16
    xlT = np.ascontiguousarray(xl.T)

    ones3 = np.ones((3, 128), dtype=np.float16)

    in_maps = []
    for c in range(N_CORES):
        sl = slice(c * BC, (c + 1) * BC)
        in_maps.append({
            "xh0": np.ascontiguousarray(xhT[:128, sl]),
            "xh1": np.ascontiguousarray(xhT[128:, sl]),
            "xl0": np.ascontiguousarray(xlT[:128, sl]),
            "xl1": np.ascontiguousarray(xlT[128:, sl]),
            "eh0": np.ascontiguousarray(eh[:128]),
            "eh1": np.ascontiguousarray(eh[128:]),
            "el0": np.ascontiguousarray(el[:128]),
            "el1": np.ascontiguousarray(el[128:]),
            "biasq": biasq,
            "ones3": ones3,
            "emb": emb,
        })
    return in_maps


def run(x, emb, trace=False, **kwargs):
    """Run the kernel; returns (out, BassKernelResults)."""
    nc = _build()
    in_maps = _prepare_inputs(x, emb)
    res = run_bass_kernel_spmd(nc, in_maps, list(range(N_CORES)),
                               trace=trace, **kwargs)
    out = np.concatenate([res.results[c]["out"] for c in range(N_CORES)], axis=0)
    return out, res


def kernel(x, emb):
    out, _ = run(x, emb, trace=False)
    return out


# revision 2
# speedup vs baseline: 1.3379x; 1.3379x over previous
"""VQ codebook lookup kernel for Trainium2 (8 NeuronCores, data-parallel).

out[b] = emb[argmin_k ||x[b] - emb[k]||^2]

Per core (8192 rows of x):
  score[b,k] = 2*x.e_k - |e_k|^2  (argmax == argmin of distance)
  computed on the PE via fp16 hi/lo split passes (x = xh+xl, 2e^T = eh+el;
  s = xh.eh + xh.el + xl.eh) which preserve fp32-grade precision.
  The -|e_k|^2 bias is folded into the xl.eh chunk for dims 128..255:
  its last 3 contraction rows carry (ones3, [q1;q2;q3]) where q1+q2+q3
  is an exact fp16 3-way split of -|e_k|^2 (drops 3 of 256 low-order
  xl-correction dims; measured harmless).
  argmax via DVE max/max_index reading PSUM, then indirect-DMA gather of
  emb rows from HBM.
"""
import os
import sys

import numpy as np

for _p in ("/opt/trn_rl_repo", "/root/.axon_site/_ro/trn_rl_repo"):
    if os.path.isdir(_p) and _p not in sys.path:
        sys.path.append(_p)

import concourse.bass as bass
import concourse.tile as tile
from concourse import bacc, mybir
from concourse.bass_utils import run_bass_kernel_spmd

N_CORES = 8
B, D, K = 65536, 256, 1024
BC = B // N_CORES            # rows per core
TILE_B = 128
N_TILES = BC // TILE_B       # 64
CHUNK = 1024                 # x load chunk width (batch cols)
N_CHUNKS = BC // CHUNK       # 8
MERGE_D = 125                # data dims kept in the merged xl chunk

f32 = mybir.dt.float32
f16 = mybir.dt.float16
u32 = mybir.dt.uint32

_nc_cache = {}


def _build():
    if "nc" in _nc_cache:
        return _nc_cache["nc"]
    nc = bacc.Bacc()

    xh0 = nc.declare_dram_parameter("xh0", [128, BC], f16, isOutput=False)
    xh1 = nc.declare_dram_parameter("xh1", [128, BC], f16, isOutput=False)
    xl0 = nc.declare_dram_parameter("xl0", [128, BC], f16, isOutput=False)
    xl1m = nc.declare_dram_parameter("xl1m", [128, BC], f16, isOutput=False)
    eh0 = nc.declare_dram_parameter("eh0", [128, K], f16, isOutput=False)
    eh1 = nc.declare_dram_parameter("eh1", [128, K], f16, isOutput=False)
    el0 = nc.declare_dram_parameter("el0", [128, K], f16, isOutput=False)
    el1 = nc.declare_dram_parameter("el1", [128, K], f16, isOutput=False)
    eh1m = nc.declare_dram_parameter("eh1m", [128, K], f16, isOutput=False)
    emb = nc.declare_dram_parameter("emb", [K, D], f32, isOutput=False)
    out = nc.declare_dram_parameter("out", [BC, D], f32, isOutput=True)

    with tile.TileContext(nc) as tc:
        with tc.tile_pool(name="res", bufs=1) as res, \
             tc.tile_pool(name="wrk", bufs=4) as wrk, \
             tc.tile_pool(name="ps", bufs=4, space="PSUM") as ps:
            teh0 = res.tile([128, K], f16, tag="eh0")
            teh1 = res.tile([128, K], f16, tag="eh1")
            tel0 = res.tile([128, K], f16, tag="el0")
            tel1 = res.tile([128, K], f16, tag="el1")
            teh1m = res.tile([128, K], f16, tag="eh1m")
            for dst, src in [(teh0, eh0), (teh1, eh1), (tel0, el0),
                             (tel1, el1), (teh1m, eh1m)]:
                nc.sync.dma_start(dst[:], src[:])

            xch = {}
            for j in range(N_CHUNKS):
                js = slice(j * CHUNK, (j + 1) * CHUNK)
                for nm, src in (("xh0", xh0), ("xh1", xh1),
                                ("xl0", xl0), ("xl1m", xl1m)):
                    t = res.tile([128, CHUNK], f16, tag=f"{nm}_{j}")
                    nc.sync.dma_start(t[:], src[:, js])
                    xch[(nm, j)] = t

            for i in range(N_TILES):
                j, c0 = divmod(i, N_CHUNKS)
                j, c0 = i // (CHUNK // TILE_B), (i % (CHUNK // TILE_B)) * TILE_B
                s = slice(c0, c0 + TILE_B)
                cxh0 = xch[("xh0", j)][:, s]
                cxh1 = xch[("xh1", j)][:, s]
                cxl0 = xch[("xl0", j)][:, s]
                cxl1m = xch[("xl1m", j)][:, s]

                psc = ps.tile([128, K], f32, tag="scores")
                h0 = psc[:, 0:512]
                h1 = psc[:, 512:1024]
                mm = nc.tensor.matmul
                # stationary-reuse order; each half is its own accum group
                mm(h0, lhsT=cxl1m, rhs=teh1m[:, 0:512], start=True, stop=False)
                mm(h1, lhsT=cxl1m, rhs=teh1m[:, 512:1024], start=True, stop=False)
                mm(h0, lhsT=cxh0, rhs=teh0[:, 0:512], start=False, stop=False)
                mm(h1, lhsT=cxh0, rhs=teh0[:, 512:1024], start=False, stop=False)
                mm(h0, lhsT=cxh0, rhs=tel0[:, 0:512], start=False, stop=False)
                mm(h1, lhsT=cxh0, rhs=tel0[:, 512:1024], start=False, stop=False)
                mm(h0, lhsT=cxh1, rhs=teh1[:, 0:512], start=False, stop=False)
                mm(h1, lhsT=cxh1, rhs=teh1[:, 512:1024], start=False, stop=False)
                mm(h0, lhsT=cxh1, rhs=tel1[:, 0:512], start=False, stop=False)
                mm(h1, lhsT=cxh1, rhs=tel1[:, 512:1024], start=False, stop=False)
                mm(h0, lhsT=cxl0, rhs=teh0[:, 0:512], start=False, stop=True)
                mm(h1, lhsT=cxl0, rhs=teh0[:, 512:1024], start=False, stop=True)

                tmax = wrk.tile([128, 8], f32, tag="maxv")
                tidx = wrk.tile([128, 8], u32, tag="idx")
                nc.vector.max(out=tmax[:], in_=psc[:])
                nc.vector.max_index(out=tidx[:], in_max=tmax[:], in_values=psc[:])

                tg = wrk.tile([128, D], f32, tag="gat")
                nc.gpsimd.indirect_dma_start(
                    out=tg[:],
                    out_offset=None,
                    in_=emb[:],
                    in_offset=bass.IndirectOffsetOnAxis(ap=tidx[:, 0:1], axis=0),
                )
                nc.sync.dma_start(out[i * TILE_B:(i + 1) * TILE_B, :], tg[:])

    nc.compile()
    _nc_cache["nc"] = nc
    return nc


def _prepare_inputs(x, emb):
    x = np.ascontiguousarray(np.asarray(x, dtype=np.float32))
    emb = np.ascontiguousarray(np.asarray(emb, dtype=np.float32))

    e2 = np.ascontiguousarray(2.0 * emb.T)              # [D, K] f32, exact
    eh = e2.astype(np.float16)
    el = (e2 - eh.astype(np.float32)).astype(np.float16)

    esq = (emb.astype(np.float64) ** 2).sum(axis=1)
    q = (-esq).astype(np.float32)
    q1 = q.astype(np.float16)
    r = q - q1.astype(np.float32)
    q2 = r.astype(np.float16)
    q3 = (r - q2.astype(np.float32)).astype(np.float16)

    xh = x.astype(np.float16)
    xl = (x - xh.astype(np.float32)).astype(np.float16)
    xhT = np.ascontiguousarray(xh.T)                    # [D, B] f16
    xlT = np.ascontiguousarray(xl.T)

    # merged chunk: 125 low-order xl/eh dims (d=128..252) + 3 bias rows
    eh1m = np.empty((128, K), np.float16)
    eh1m[:MERGE_D] = eh[128:128 + MERGE_D]
    eh1m[125] = q1
    eh1m[126] = q2
    eh1m[127] = q3
    eh1m = np.ascontiguousarray(eh1m)

    xl1m_full = np.empty((128, B), np.float16)
    xl1m_full[:MERGE_D] = xlT[128:128 + MERGE_D]
    xl1m_full[125:128] = 1.0

    in_maps = []
    for c in range(N_CORES):
        sl = slice(c * BC, (c + 1) * BC)
        in_maps.append({
            "xh0": np.ascontiguousarray(xhT[:128, sl]),
            "xh1": np.ascontiguousarray(xhT[128:, sl]),
            "xl0": np.ascontiguousarray(xlT[:128, sl]),
            "xl1m": np.ascontiguousarray(xl1m_full[:, sl]),
            "eh0": np.ascontiguousarray(eh[:128]),
            "eh1": np.ascontiguousarray(eh[128:]),
            "el0": np.ascontiguousarray(el[:128]),
            "el1": np.ascontiguousarray(el[128:]),
            "eh1m": eh1m,
            "emb": emb,
        })
    return in_maps


def run(x, emb, trace=False, **kwargs):
    """Run the kernel; returns (out, BassKernelResults)."""
    nc = _build()
    in_maps = _prepare_inputs(x, emb)
    res = run_bass_kernel_spmd(nc, in_maps, list(range(N_CORES)),
                               trace=trace, **kwargs)
    out = np.concatenate([res.results[c]["out"] for c in range(N_CORES)], axis=0)
    return out, res


def kernel(x, emb):
    out, _ = run(x, emb, trace=False)
    return out
